# revision 1
# baseline (speedup 1.0000x reference)
"""DeepSet-equivariant layer on 8 TRN2 NeuronCores.

Math (reference):
    y = x @ w1 + (colsum(x) @ w2) / n + bias        x: (n, 128)

Distribution strategy (sharding_hint): shard x and y along the set
dimension n across the 8 cores; each core computes its local column-sum,
an AllReduce produces the global pooled vector, and w1/w2/bias are
replicated.

Device layout trick: each core receives its shard PRE-TRANSPOSED on the
host as xT (128=d_in partitions, rows free) and returns yT in the same
layout.  This makes the kernel transpose-free:
  - main matmul: lhsT = w1 (stationary), rhs = xT chunk -> yT in PSUM
  - column-sum  = free-dim reduce of xT
  - transmit+bias is a per-partition column in yT layout, so the PSUM
    drain and the transmit add fuse into one tensor_scalar/activation.

Schedule: phase 1 streams all of xT into a resident SBUF buffer (pure
DMA-bound; the per-chunk column-sum reduces hide under the DMA), then
the pooled vector is AllReduced (a warm-up collective issued at kernel
start has already absorbed the ncfw wake + cross-core barrier), then
phase 2 matmuls from resident x, fuses +transmit into the PSUM drain,
and streams the result out.

Matmuls run in float32r (same 4-byte storage, 1 cycle/row on the PE
array vs 4 for strict fp32, ~1e-4 relative error).
"""

import numpy as np

import concourse.bass as bass
import concourse.tile as tile
from concourse import bacc, mybir
from concourse.bass_utils import run_bass_kernel_spmd

N_CORES = 8
D = 128                 # d_in == d_out
N_ROWS = 200000         # full set size
R = 25088               # padded rows per core: 8 * 25088 = 200704 >= 200000
IN_CHUNK = 4096         # columns per input DMA chunk (2 MiB)
OUT_CHUNK = 2048        # columns per PSUM tile / fused-add / output DMA (1 MiB)
MM_N = 512              # moving-operand free dim per matmul

F32 = mybir.dt.float32
F32R = mybir.dt.float32r


def _split(r, step):
    out = []
    c0 = 0
    while c0 < r:
        cw = min(step, r - c0)
        out.append((c0, cw))
        c0 += cw
    return out


def build_nc(r: int, n_total: int, mm_dtype=F32R):
    """Build the SPMD Bass program for one core holding r rows."""
    in_chunks = _split(r, IN_CHUNK)
    out_chunks = _split(r, OUT_CHUNK)

    nc = bacc.Bacc(
        "TRN2",
        target_bir_lowering=False,
        debug=False,
        num_devices=N_CORES,
    )

    xt = nc.declare_dram_parameter("xt", [D, r], mm_dtype, isOutput=False)
    w1 = nc.declare_dram_parameter("w1", [D, D], mm_dtype, isOutput=False)
    w2 = nc.declare_dram_parameter("w2", [D, D], F32, isOutput=False)
    bias_c = nc.declare_dram_parameter("bias_c", [D, 1], F32, isOutput=False)
    out = nc.declare_dram_parameter("out", [D, r], F32, isOutput=True)

    # Bounce buffers for the collective (collectives cannot touch I/O tensors).
    cc_in = nc.dram_tensor("cc_in", [D, 1], F32)
    cc_out = nc.dram_tensor("cc_out", [D, 1], F32, addr_space="Shared")
    ccw_in = nc.dram_tensor("ccw_in", [D, 1], F32)
    ccw_out = nc.dram_tensor("ccw_out", [D, 1], F32, addr_space="Shared")

    # Warm-up collective, emitted BEFORE the TileContext so it is the very
    # first gpsimd instruction: wakes ncfw and enters the cross-core
    # barrier immediately at kernel start (Tile's startup barrier would
    # otherwise delay it ~10us).  Content/result unused; completion is
    # guaranteed before the real AllReduce because ncfw serializes
    # collectives in program order.
    warm_sem = nc.alloc_semaphore("warm_cc")
    nc.gpsimd.collective_compute(
        "AllReduce",
        mybir.AluOpType.add,
        replica_groups=[list(range(N_CORES))],
        ins=[ccw_in.ap().opt()],
        outs=[ccw_out.ap().opt()],
    ).then_inc(warm_sem)

    with tile.TileContext(nc) as tc:
        with (
            tc.tile_pool(name="const", bufs=1) as const_pool,
            tc.tile_pool(name="xres", bufs=1) as xres_pool,
            tc.tile_pool(name="obuf", bufs=4) as obuf_pool,
            tc.tile_pool(name="small", bufs=1) as small_pool,
            tc.tile_pool(name="mm", bufs=2, space=bass.MemorySpace.PSUM) as mm_pool,
        ):
            w1_sb = const_pool.tile([D, D], mm_dtype)
            w2_sb = const_pool.tile([D, D], F32)
            bias_sb = const_pool.tile([D, 1], F32)
            nc.sync.dma_start(w1_sb[:], w1[:, :])
            nc.sync.dma_start(w2_sb[:], w2[:, :])
            nc.sync.dma_start(bias_sb[:], bias_c[:, :])

            # phase 1: stream ALL of xT into a resident SBUF buffer; the
            # column-sum reduces (split DVE/ACT) hide under the DMA.
            x_sb = xres_pool.tile([D, r], mm_dtype)
            n_in = len(in_chunks)
            cs_parts = small_pool.tile([D, n_in], F32)
            trash = small_pool.tile([D, IN_CHUNK], mybir.dt.bfloat16)

            for c, (c0, cw) in enumerate(in_chunks):
                # alternate the two HWDGE rings (SP + ACT) for more DMA overlap
                dma_eng = nc.sync if c % 2 == 0 else nc.scalar
                dma_eng.dma_start(x_sb[:, c0 : c0 + cw], xt[:, c0 : c0 + cw])
                if c % 2 == 0 or c == n_in - 1:
                    nc.vector.reduce_sum(
                        cs_parts[:, c : c + 1],
                        x_sb[:, c0 : c0 + cw].bitcast(F32),
                        axis=mybir.AxisListType.X,
                    )
                else:
                    nc.scalar.activation(
                        trash[:, :cw],
                        x_sb[:, c0 : c0 + cw].bitcast(F32),
                        mybir.ActivationFunctionType.Copy,
                        accum_out=cs_parts[:, c : c + 1],
                    )

            # local pooled vector -> AllReduce
            cs = small_pool.tile([D, 1], F32)
            nc.vector.reduce_sum(cs[:], cs_parts[:], axis=mybir.AxisListType.X)
            nc.sync.dma_start(cc_in[:, :], cs[:])

            nc.gpsimd.collective_compute(
                "AllReduce",
                mybir.AluOpType.add,
                replica_groups=[list(range(N_CORES))],
                ins=[cc_in.ap().opt()],
                outs=[cc_out.ap().opt()],
            )

            gcs = small_pool.tile([D, 1], F32)
            nc.sync.dma_start(gcs[:], cc_out[:, :])

            # transmit column: t = (w2.T @ gcs) / n + bias
            t_ps = mm_pool.tile([D, OUT_CHUNK], F32, tag="ps")
            nc.tensor.matmul(t_ps[:, :1], w2_sb[:], gcs[:])
            t_sb = small_pool.tile([D, 1], F32)
            nc.vector.tensor_scalar(
                out=t_sb[:],
                in0=t_ps[:, :1],
                scalar1=1.0 / float(n_total),
                scalar2=bias_sb[:],
                op0=mybir.AluOpType.mult,
                op1=mybir.AluOpType.add,
            )

            # phase 2: matmul from resident x, fuse +transmit into the PSUM
            # drain (alternating DVE/ACT), stream out.
            for c, (c0, cw) in enumerate(out_chunks):
                ps = mm_pool.tile([D, OUT_CHUNK], F32, tag="ps")
                s0 = 0
                while s0 < cw:
                    sw = min(MM_N, cw - s0)
                    nc.tensor.matmul(
                        ps[:, s0 : s0 + sw],
                        w1_sb[:],
                        x_sb[:, c0 + s0 : c0 + s0 + sw],
                    )
                    s0 += sw
                ob = obuf_pool.tile([D, OUT_CHUNK], F32)
                if c % 2 == 0:
                    nc.vector.tensor_scalar(
                        out=ob[:, :cw],
                        in0=ps[:, :cw],
                        scalar1=t_sb[:],
                        scalar2=None,
                        op0=mybir.AluOpType.add,
                    )
                else:
                    nc.scalar.activation(
                        ob[:, :cw],
                        ps[:, :cw],
                        mybir.ActivationFunctionType.Identity,
                        bias=t_sb[:],
                        scale=1.0,
                    )
                (nc.sync if c % 2 == 0 else nc.scalar).dma_start(
                    out[:, c0 : c0 + cw], ob[:, :cw]
                )

    nc.compile()
    return nc


_nc_cache: dict = {}


def _get_nc(r: int, n_total: int):
    key = (r, n_total)
    if key not in _nc_cache:
        _nc_cache[key] = build_nc(r, n_total)
    return _nc_cache[key]


LAST_RESULTS = None


def _execute(x, w1, w2, bias, r, trace=False, tmpdir=None, trace_cores=None):
    global LAST_RESULTS
    x = np.ascontiguousarray(np.asarray(x, dtype=np.float32))
    w1 = np.ascontiguousarray(np.asarray(w1, dtype=np.float32))
    w2 = np.ascontiguousarray(np.asarray(w2, dtype=np.float32))
    bias = np.asarray(bias, dtype=np.float32)
    n, d = x.shape
    assert d == D and r * N_CORES >= n

    xp = np.zeros((N_CORES * r, d), dtype=np.float32)
    xp[:n] = x
    # (8, r, d) -> (8, d, r) pre-transposed shards
    xts = np.ascontiguousarray(xp.reshape(N_CORES, r, d).transpose(0, 2, 1))
    bias_col = np.ascontiguousarray(bias.reshape(1, d).T)

    in_maps = [
        {"xt": xts[i], "w1": w1, "w2": w2, "bias_c": bias_col}
        for i in range(N_CORES)
    ]

    nc = _get_nc(r, n)
    kwargs = {}
    if trace:
        kwargs.update(trace=True, tmpdir=tmpdir)
        if trace_cores is not None:
            kwargs.update(trace_cores=trace_cores)
    res = run_bass_kernel_spmd(nc, in_maps, core_ids=list(range(N_CORES)), **kwargs)
    LAST_RESULTS = res

    yts = [res.results[i]["out"] for i in range(N_CORES)]  # each (D, r)
    y = np.concatenate([yt.T for yt in yts], axis=0)[:n]
    return np.ascontiguousarray(y)


def kernel(x, w1, w2, bias):
    return _execute(x, w1, w2, bias, R)



# revision 2
# speedup vs baseline: 1.6365x; 1.6365x over previous
"""DeepSet-equivariant layer on 8 TRN2 NeuronCores.

Math (reference):
    y = x @ w1 + (colsum(x) @ w2) / n + bias        x: (n, 128)

Distribution strategy (sharding_hint): shard x and y along the set
dimension n across the 8 cores; each core computes its local column-sum,
an AllReduce produces the global pooled vector, and w1/w2/bias are
replicated.

Device layout trick: each core receives its shard PRE-TRANSPOSED on the
host as xT (128=d_in partitions, rows free) and returns yT in the same
layout.  This makes the kernel transpose-free:
  - main matmul: lhsT = w1 (stationary), rhs = xT chunk -> yT in PSUM
  - column-sum  = free-dim reduce of xT
  - transmit+bias is a per-partition column in yT layout, so the PSUM
    drain and the transmit add fuse into one tensor_scalar/activation.

The x / w1 / y streams are bf16 (host casts both ways; rel err ~3e-3,
well inside the 2e-2 gate) which HALVES the HBM traffic per core:
6.4 MB in + 6.4 MB out at ~350 GB/s ~= 37 us of DMA, vs 74 us in fp32.
The pooled vector, w2, bias and all accumulation stay fp32.

Schedule: a warm-up collective issued before the TileContext absorbs the
ncfw wake + cross-core barrier while phase 1 streams x into a resident
SBUF buffer (column-sum reduces hide under the DMA).  The 128-float
pooled vector is AllReduced, then phase 2 matmuls from resident x, fuses
the +transmit into the PSUM drain (alternating DVE/ACT), and streams the
bf16 result out in 2 MB chunks.
"""

import numpy as np
import ml_dtypes

import concourse.bass as bass
import concourse.tile as tile
from concourse import bacc, mybir
from concourse.bass_utils import run_bass_kernel_spmd

N_CORES = 8
D = 128                 # d_in == d_out
N_ROWS = 200000         # full set size
R = 25088               # padded rows per core: 8 * 25088 = 200704 >= 200000
IN_CHUNK = 8192         # columns per input DMA chunk (2 MiB bf16)
OUT_CHUNK = 8192        # columns per output DMA chunk (2 MiB bf16)
MM_N = 512              # moving-operand free dim per matmul (one PSUM bank)

F32 = mybir.dt.float32
BF16 = mybir.dt.bfloat16
NP_BF16 = ml_dtypes.bfloat16


def _split(r, step):
    out = []
    c0 = 0
    while c0 < r:
        cw = min(step, r - c0)
        out.append((c0, cw))
        c0 += cw
    return out


def build_nc(r: int, n_total: int):
    """Build the SPMD Bass program for one core holding r rows."""
    in_chunks = _split(r, IN_CHUNK)
    out_chunks = _split(r, OUT_CHUNK)

    nc = bacc.Bacc(
        "TRN2",
        target_bir_lowering=False,
        debug=False,
        num_devices=N_CORES,
    )

    xt = nc.declare_dram_parameter("xt", [D, r], BF16, isOutput=False)
    w1 = nc.declare_dram_parameter("w1", [D, D], BF16, isOutput=False)
    w2 = nc.declare_dram_parameter("w2", [D, D], F32, isOutput=False)
    bias_c = nc.declare_dram_parameter("bias_c", [D, 1], F32, isOutput=False)
    out = nc.declare_dram_parameter("out", [D, r], BF16, isOutput=True)

    # Bounce buffers for the collective (collectives cannot touch I/O tensors).
    cc_in = nc.dram_tensor("cc_in", [D, 1], F32)
    cc_out = nc.dram_tensor("cc_out", [D, 1], F32, addr_space="Shared")
    ccw_in = nc.dram_tensor("ccw_in", [D, 1], F32)
    ccw_out = nc.dram_tensor("ccw_out", [D, 1], F32, addr_space="Shared")

    # Warm-up collective, emitted BEFORE the TileContext so it is the very
    # first gpsimd instruction: wakes ncfw and enters the cross-core
    # barrier immediately at kernel start.  Content/result unused;
    # completion is guaranteed before the real AllReduce because ncfw
    # serializes collectives in program order.
    warm_sem = nc.alloc_semaphore("warm_cc")
    nc.gpsimd.collective_compute(
        "AllReduce",
        mybir.AluOpType.add,
        replica_groups=[list(range(N_CORES))],
        ins=[ccw_in.ap().opt()],
        outs=[ccw_out.ap().opt()],
    ).then_inc(warm_sem)

    with tile.TileContext(nc) as tc:
        with (
            tc.tile_pool(name="const", bufs=1) as const_pool,
            tc.tile_pool(name="xres", bufs=1) as xres_pool,
            tc.tile_pool(name="obuf", bufs=3) as obuf_pool,
            tc.tile_pool(name="small", bufs=1) as small_pool,
            tc.tile_pool(name="mm", bufs=8, space=bass.MemorySpace.PSUM) as mm_pool,
        ):
            w1_sb = const_pool.tile([D, D], BF16)
            w2_sb = const_pool.tile([D, D], F32)
            bias_sb = const_pool.tile([D, 1], F32)
            nc.scalar.dma_start(w1_sb[:], w1[:, :])
            nc.scalar.dma_start(w2_sb[:], w2[:, :])
            nc.scalar.dma_start(bias_sb[:], bias_c[:, :])

            # phase 1: stream ALL of xT into a resident SBUF buffer; the
            # per-chunk column-sum reduces (DVE) hide under the DMA.
            x_sb = xres_pool.tile([D, r], BF16)
            n_in = len(in_chunks)
            cs_parts = small_pool.tile([D, n_in], F32)

            for c, (c0, cw) in enumerate(in_chunks):
                # alternate the two HWDGE rings (SP + ACT) for more overlap
                dma_eng = nc.sync if c % 2 == 0 else nc.scalar
                dma_eng.dma_start(x_sb[:, c0 : c0 + cw], xt[:, c0 : c0 + cw])
                nc.vector.reduce_sum(
                    cs_parts[:, c : c + 1],
                    x_sb[:, c0 : c0 + cw],
                    axis=mybir.AxisListType.X,
                )

            # local pooled vector -> AllReduce
            cs = small_pool.tile([D, 1], F32)
            nc.vector.reduce_sum(cs[:], cs_parts[:], axis=mybir.AxisListType.X)
            nc.sync.dma_start(cc_in[:, :], cs[:])

            nc.gpsimd.collective_compute(
                "AllReduce",
                mybir.AluOpType.add,
                replica_groups=[list(range(N_CORES))],
                ins=[cc_in.ap().opt()],
                outs=[cc_out.ap().opt()],
            )

            gcs = small_pool.tile([D, 1], F32)
            nc.sync.dma_start(gcs[:], cc_out[:, :])

            # transmit column: t = (w2.T @ gcs) / n + bias
            t_ps = mm_pool.tile([D, MM_N], F32, tag="ps")
            nc.tensor.matmul(t_ps[:, :1], w2_sb[:], gcs[:])
            t_sb = small_pool.tile([D, 1], F32)
            nc.vector.tensor_scalar(
                out=t_sb[:],
                in0=t_ps[:, :1],
                scalar1=1.0 / float(n_total),
                scalar2=bias_sb[:],
                op0=mybir.AluOpType.mult,
                op1=mybir.AluOpType.add,
            )

            # phase 2: matmul from resident x into one PSUM bank at a time,
            # fuse +transmit into the PSUM drain (alternating DVE/ACT),
            # stream the bf16 result out in 2 MB chunks.
            k = 0
            for c, (c0, cw) in enumerate(out_chunks):
                ob = obuf_pool.tile([D, OUT_CHUNK], BF16)
                s0 = 0
                while s0 < cw:
                    sw = min(MM_N, cw - s0)
                    ps = mm_pool.tile([D, MM_N], F32, tag="ps")
                    nc.tensor.matmul(
                        ps[:, :sw],
                        w1_sb[:],
                        x_sb[:, c0 + s0 : c0 + s0 + sw],
                    )
                    if k % 2 == 0:
                        nc.vector.tensor_scalar(
                            out=ob[:, s0 : s0 + sw],
                            in0=ps[:, :sw],
                            scalar1=t_sb[:],
                            scalar2=None,
                            op0=mybir.AluOpType.add,
                        )
                    else:
                        nc.scalar.activation(
                            ob[:, s0 : s0 + sw],
                            ps[:, :sw],
                            mybir.ActivationFunctionType.Identity,
                            bias=t_sb[:],
                            scale=1.0,
                        )
                    k += 1
                    s0 += sw
                (nc.sync if c % 2 == 0 else nc.scalar).dma_start(
                    out[:, c0 : c0 + cw], ob[:, :cw]
                )

    nc.compile()
    return nc


_nc_cache: dict = {}


def _get_nc(r: int, n_total: int):
    key = (r, n_total)
    if key not in _nc_cache:
        _nc_cache[key] = build_nc(r, n_total)
    return _nc_cache[key]


LAST_RESULTS = None


def _execute(x, w1, w2, bias, r, trace=False, tmpdir=None, trace_cores=None):
    global LAST_RESULTS
    x = np.ascontiguousarray(np.asarray(x, dtype=np.float32))
    w1 = np.asarray(w1, dtype=np.float32)
    w2 = np.ascontiguousarray(np.asarray(w2, dtype=np.float32))
    bias = np.asarray(bias, dtype=np.float32)
    n, d = x.shape
    assert d == D and r * N_CORES >= n

    xp = np.zeros((N_CORES * r, d), dtype=np.float32)
    xp[:n] = x
    # (8, r, d) -> (8, d, r) pre-transposed bf16 shards
    xts = np.ascontiguousarray(
        xp.reshape(N_CORES, r, d).transpose(0, 2, 1).astype(NP_BF16)
    )
    w1_b = np.ascontiguousarray(w1.astype(NP_BF16))
    bias_col = np.ascontiguousarray(bias.reshape(1, d).T)

    in_maps = [
        {"xt": xts[i], "w1": w1_b, "w2": w2, "bias_c": bias_col}
        for i in range(N_CORES)
    ]

    nc = _get_nc(r, n)
    kwargs = {}
    if trace:
        kwargs.update(trace=True, tmpdir=tmpdir)
        if trace_cores is not None:
            kwargs.update(trace_cores=trace_cores)
    res = run_bass_kernel_spmd(nc, in_maps, core_ids=list(range(N_CORES)), **kwargs)
    LAST_RESULTS = res

    yts = [res.results[i]["out"] for i in range(N_CORES)]  # each (D, r) bf16
    y = np.concatenate([yt.T.astype(np.float32) for yt in yts], axis=0)[:n]
    return np.ascontiguousarray(y)


def kernel(x, w1, w2, bias):
    return _execute(x, w1, w2, bias, R)
